# revision 1
# baseline (speedup 1.0000x reference)
"""Trainium2 Bass kernel: 7x7 valid cross-correlation (Conv2D) + bias on a
4096x4096 fp32 image, row-sharded over 8 NeuronCores (512 output rows each,
with a 6-row halo included in each core's input slice).

Algorithm per core:
  - Output rows are processed in tiles of 122 (=128-6) rows.
  - For each row-tile and each 512-wide output column chunk, the 2D conv is
    computed as 7 accumulating TensorE matmuls (one per horizontal tap b):
        psum[m, n] += B_b.T @ x[:, n+b]
    where B_b[k, m] = w[k-m, b] is a banded [128 x 122] matrix that performs
    the 7-tap vertical convolution for kernel column b.
  - PSUM is evacuated by the VectorE with a fused bias add (tensor_scalar_add).
Weight/bias (banded matrices) are built host-side and replicated to all cores.
"""

import sys

sys.path.insert(0, "/opt/trn_rl_repo")

import numpy as np

import concourse.bass as bass
import concourse.bacc as bacc
import concourse.mybir as mybir
from concourse.tile import TileContext
from concourse.bass_utils import run_bass_kernel_spmd

KH, KW = 7, 7
H, W = 4096, 4096
OH, OW = H - KH + 1, W - KW + 1  # 4090, 4090

# 1x8 core grid: rows sharded across all 8 cores. Full-width tiles keep the
# per-partition DMA descriptor at 16KB (a whole image row) -- per-descriptor
# cost caps DMA throughput, so wide rows matter more than fewer PE cycles.
RB, CB = 8, 1
CORE_OR, CORE_OC = 512, OW             # per-core output shape (rows padded)
CORE_IR, CORE_IC = CORE_OR + KH - 1, W  # 518, 4096
TILE_R = 128 - (KH - 1)                # 122 output rows per row-tile
CHUNK = 512                            # output cols per PSUM bank (fp32)

_NC_CACHE = {}


def _build_nc(core_or, core_oc, core_ir, core_ic, tile_r, chunk):
    """Build the single-core Bass program (SPMD: same program on all cores)."""
    f32 = mybir.dt.float32
    # float32r: same 4-byte layout as fp32 but the PE streams it at 1
    # cycle/row (vs 4 for true fp32) when the moving dim is >=256.
    f32r = mybir.dt.float32r
    kin = tile_r + KH - 1  # input rows per full tile (<=128)
    assert kin <= 128
    n_tiles = -(-core_or // tile_r)
    n_chunks = -(-core_oc // chunk)

    nc = bacc.Bacc()
    x_in = nc.declare_dram_parameter("x_in", [core_ir, core_ic], f32, isOutput=False)
    # bands go through the (slow, but tiny and one-off) f32r DMA path so the
    # verifier sees them as fp32r-rounded.
    bands = nc.declare_dram_parameter("bands", [kin, KW * tile_r], f32r, isOutput=False)
    biasb = nc.declare_dram_parameter("biasb", [128, 1], f32, isOutput=False)
    # Output rows are padded to a 32B-aligned stride: a 4090-float (16360B)
    # row stride makes every other row start unaligned, which drops the
    # write DMA to 16B elements (~50 GB/s). The host slices off the pad.
    oc_pad = -(-core_oc // 16) * 16  # 4096
    y_out = nc.declare_dram_parameter("y_out", [core_or, oc_pad], f32, isOutput=True)

    with TileContext(nc) as tc:
        with (
            tc.tile_pool(name="const", bufs=1) as cpool,
            tc.tile_pool(name="io", bufs=3) as iopool,
            tc.tile_pool(name="ps", bufs=8, space="PSUM") as ppool,
        ):
            band_sb = cpool.tile([kin, KW * tile_r], f32r)
            bias_sb = cpool.tile([128, 1], f32)

            for t in range(n_tiles):
                r0 = t * tile_r
                h = min(tile_r, core_or - r0)
                kh = h + KH - 1
                x_sb = iopool.tile([kin, core_ic], f32, tag="x")
                nc.sync.dma_start(out=x_sb[:kh, :], in_=x_in[r0 : r0 + kh, :])
                if t == 0:
                    # consts issued after the first x row-block so the
                    # critical-path load starts immediately
                    nc.sync.dma_start(out=band_sb[:, :], in_=bands[:, :])
                    nc.sync.dma_start(out=bias_sb[:, :], in_=biasb[:, :])
                # fp32r matmul operands must be explicitly rounded; a DVE
                # copy-cast does it at on-chip rate while the bulk DMA stays
                # on the fast plain-f32 path.
                x_r = iopool.tile([kin, core_ic], f32r, tag="xr")
                nc.vector.tensor_copy(x_r[:kh, :], x_sb[:kh, :])
                y_sb = iopool.tile([128, core_oc], f32, tag="y")
                for j in range(n_chunks):
                    c0 = j * chunk
                    cw = min(chunk, core_oc - c0)
                    ps = ppool.tile([128, chunk], f32, tag="ps")
                    for b in range(KW):
                        nc.tensor.matmul(
                            ps[:h, :cw],
                            lhsT=band_sb[:kh, b * tile_r : b * tile_r + h],
                            rhs=x_r[:kh, c0 + b : c0 + b + cw],
                            start=(b == 0),
                            stop=(b == KW - 1),
                        )
                    nc.vector.tensor_scalar_add(
                        y_sb[:h, c0 : c0 + cw], ps[:h, :cw], bias_sb[:h, 0:1]
                    )
                nc.sync.dma_start(
                    out=y_out[r0 : r0 + h, :core_oc], in_=y_sb[:h, :]
                )
    nc.compile()
    return nc


def _make_bands(weight, tile_r):
    """B_b[k, m] = w[k-m, b] laid out as [kin, KW*tile_r] (band b in cols
    [b*tile_r, (b+1)*tile_r))."""
    kin = tile_r + KH - 1
    bands = np.zeros((kin, KW * tile_r), np.float32)
    m = np.arange(tile_r)
    for b in range(KW):
        for a in range(KH):
            bands[m + a, b * tile_r + m] = weight[a, b]
    return bands


def _shard_inputs(x, weight, bias):
    bands = _make_bands(weight, TILE_R)
    biasb = np.full((128, 1), np.float32(bias[0]), np.float32)
    in_maps = []
    for rb in range(RB):
        for cb in range(CB):
            r0, c0 = rb * CORE_OR, cb * CORE_OC
            rr = min(CORE_IR, H - r0)
            cc = min(CORE_IC, W - c0)
            xt = np.zeros((CORE_IR, CORE_IC), np.float32)
            xt[:rr, :cc] = x[r0 : r0 + rr, c0 : c0 + cc]
            in_maps.append({"x_in": xt, "bands": bands, "biasb": biasb})
    return in_maps


def _assemble(results):
    out = np.empty((OH, OW), np.float32)
    i = 0
    for rb in range(RB):
        for cb in range(CB):
            r0, c0 = rb * CORE_OR, cb * CORE_OC
            rr = min(CORE_OR, OH - r0)
            cc = min(CORE_OC, OW - c0)
            out[r0 : r0 + rr, c0 : c0 + cc] = results[i]["y_out"][:rr, :cc]  # drops row pad
            i += 1
    return out


def _get_nc():
    key = (CORE_OR, CORE_OC, TILE_R, CHUNK)
    if key not in _NC_CACHE:
        _NC_CACHE[key] = _build_nc(CORE_OR, CORE_OC, CORE_IR, CORE_IC, TILE_R, CHUNK)
    return _NC_CACHE[key]


def _run(x, weight, bias, **spmd_kwargs):
    x = np.ascontiguousarray(np.asarray(x), dtype=np.float32)
    weight = np.asarray(weight, dtype=np.float32)
    bias = np.asarray(bias, dtype=np.float32)
    in_maps = _shard_inputs(x, weight, bias)
    res = run_bass_kernel_spmd(_get_nc(), in_maps, list(range(RB * CB)), **spmd_kwargs)
    return _assemble(res.results), res


def kernel(x, weight, bias):
    out, _ = _run(x, weight, bias)
    return out



# revision 2
# speedup vs baseline: 1.6462x; 1.6462x over previous
"""Trainium2 Bass kernel: 7x7 valid cross-correlation (Conv2D) + bias on a
4096x4096 fp32 image, column-sharded over 8 NeuronCores (512 output columns
each, with a 6-column halo in each core's input slice).

Key design points (v2):
  - Column sharding: each core sees all 4096 input rows x 518 cols. Row
    tiles of 122 output rows (kin=128 input rows) give 34 tiles with only
    1.4% quantization waste (vs 19% for the row-sharded 512/122 split).
  - Input is cast to bf16 host-side: halves input HBM traffic and the PE
    streams bf16 at 1 col/cycle with no on-device f32r cast.
  - The 2D conv is 7 accumulating matmuls per tile (one per horizontal tap
    b): psum[m, n] += B_b.T @ x[:, n+b], with B_b[k, m] = w[k-m, b] a
    banded [128 x 122] matrix doing the 7-tap vertical convolution.
  - DMA partition counts are chosen for SDMA engine spray: the HWDGE
    splits a transfer's per-partition descriptors across
    (largest divisor of partition count <= 16) engines at ~27 GB/s each.
    122 = 2*61 would serialize on 2 engines, so input tiles load 128 rows
    and output writes are split 80+42 (16- and 14-way spray).
  - Output stays fp32 (write bandwidth has headroom; halves rounding err).
"""

import sys

sys.path.insert(0, "/opt/trn_rl_repo")

import numpy as np

import concourse.bass as bass
import concourse.bacc as bacc
import concourse.mybir as mybir
from concourse.tile import TileContext
from concourse.bass_utils import run_bass_kernel_spmd

KH, KW = 7, 7
H, W = 4096, 4096
OH, OW = H - KH + 1, W - KW + 1  # 4090, 4090

NCORES = 8
CORE_OC = 512                     # output cols per core (core 7: 506 valid)
CORE_IC = CORE_OC + KW - 1        # 518 input cols per core
TILE_R = 122                      # output rows per tile (kin = 128)
KIN = TILE_R + KH - 1             # 128
N_TILES = -(-OH // TILE_R)        # 34 (33x122 + 64)
OSPLIT = 80                       # output write split: 80 (16 eng) + 42 (14 eng)

BF16 = None  # set below

_NC_CACHE = {}


def _build_nc():
    f32 = mybir.dt.float32
    bf16 = mybir.dt.bfloat16

    nc = bacc.Bacc()
    x_in = nc.declare_dram_parameter("x_in", [H, CORE_IC], bf16, isOutput=False)
    bands = nc.declare_dram_parameter("bands", [KIN, KW * TILE_R], bf16, isOutput=False)
    biasb = nc.declare_dram_parameter("biasb", [128, 1], f32, isOutput=False)
    y_out = nc.declare_dram_parameter("y_out", [OH, CORE_OC], f32, isOutput=True)

    with TileContext(nc) as tc:
        with (
            tc.tile_pool(name="const", bufs=1) as cpool,
            tc.tile_pool(name="io", bufs=4) as iopool,
            tc.tile_pool(name="ps", bufs=4, space="PSUM") as ppool,
        ):
            band_sb = cpool.tile([KIN, KW * TILE_R], bf16)
            bias_sb = cpool.tile([128, 1], f32)

            for t in range(N_TILES):
                r0 = t * TILE_R
                h = min(TILE_R, OH - r0)
                kh = h + KH - 1
                x_sb = iopool.tile([KIN, CORE_IC], bf16, tag="x")
                nc.sync.dma_start(out=x_sb[:kh, :], in_=x_in[r0 : r0 + kh, :])
                if t == 0:
                    # consts issued after the first x row-block so the
                    # critical-path load starts immediately
                    nc.sync.dma_start(out=band_sb[:, :], in_=bands[:, :])
                    nc.sync.dma_start(out=bias_sb[:, :], in_=biasb[:, :])
                ps = ppool.tile([128, CORE_OC], f32, tag="ps")
                for b in range(KW):
                    nc.tensor.matmul(
                        ps[:h, :],
                        lhsT=band_sb[:kh, b * TILE_R : b * TILE_R + h],
                        rhs=x_sb[:kh, b : b + CORE_OC],
                        start=(b == 0),
                        stop=(b == KW - 1),
                    )
                y_sb = iopool.tile([128, CORE_OC], f32, tag="y")
                nc.vector.tensor_scalar_add(y_sb[:h, :], ps[:h, :], bias_sb[:h, 0:1])
                if h == TILE_R:
                    nc.sync.dma_start(
                        out=y_out[r0 : r0 + OSPLIT, :], in_=y_sb[:OSPLIT, :]
                    )
                    nc.sync.dma_start(
                        out=y_out[r0 + OSPLIT : r0 + h, :], in_=y_sb[OSPLIT:h, :]
                    )
                else:
                    nc.sync.dma_start(out=y_out[r0 : r0 + h, :], in_=y_sb[:h, :])
    nc.compile()
    return nc


def _make_bands(weight):
    """B_b[k, m] = w[k-m, b] laid out as [KIN, KW*TILE_R] (band b in cols
    [b*TILE_R, (b+1)*TILE_R))."""
    bands = np.zeros((KIN, KW * TILE_R), np.float32)
    m = np.arange(TILE_R)
    for b in range(KW):
        for a in range(KH):
            bands[m + a, b * TILE_R + m] = weight[a, b]
    return bands.astype(mybir.dt.np(mybir.dt.bfloat16))


def _shard_inputs(x, weight, bias):
    bands = _make_bands(weight)
    biasb = np.full((128, 1), np.float32(bias[0]), np.float32)
    xb = x.astype(mybir.dt.np(mybir.dt.bfloat16))
    in_maps = []
    for c in range(NCORES):
        c0 = c * CORE_OC
        cc = min(CORE_IC, W - c0)
        xt = np.zeros((H, CORE_IC), xb.dtype)
        xt[:, :cc] = xb[:, c0 : c0 + cc]
        in_maps.append({"x_in": xt, "bands": bands, "biasb": biasb})
    return in_maps


def _assemble(results):
    out = np.empty((OH, OW), np.float32)
    for c in range(NCORES):
        c0 = c * CORE_OC
        cc = min(CORE_OC, OW - c0)
        out[:, c0 : c0 + cc] = results[c]["y_out"][:, :cc]
    return out


def _get_nc():
    if "nc" not in _NC_CACHE:
        _NC_CACHE["nc"] = _build_nc()
    return _NC_CACHE["nc"]


def _run(x, weight, bias, **spmd_kwargs):
    x = np.ascontiguousarray(np.asarray(x), dtype=np.float32)
    weight = np.asarray(weight, dtype=np.float32)
    bias = np.asarray(bias, dtype=np.float32)
    in_maps = _shard_inputs(x, weight, bias)
    res = run_bass_kernel_spmd(_get_nc(), in_maps, list(range(NCORES)), **spmd_kwargs)
    return _assemble(res.results), res


def kernel(x, weight, bias):
    out, _ = _run(x, weight, bias)
    return out


# revision 3
# speedup vs baseline: 2.7505x; 1.6708x over previous
"""Trainium2 Bass kernel: 7x7 valid cross-correlation (Conv2D) + bias on a
4096x4096 fp32 image, column-sharded over 8 NeuronCores (512 output columns
each, with a 6-column halo in each core's input slice).

Design (v3):
  - Column sharding: each core sees all 4096 input rows x 518 cols. Row
    tiles of 122 output rows (kin=128 input rows) give 34 tiles with only
    1.4% quantization waste (vs 19% for a row-sharded 512/122 split).
  - Input is cast to bf16 host-side: halves input HBM traffic and the PE
    streams bf16 at 1 col/cycle with no on-device cast.
  - The 2D conv is 7 accumulating matmuls per tile (one per horizontal tap
    b): psum[m, n] += B_b.T @ x[:, n+b], with B_b[k, m] = w[k-m, b] a
    banded [128 x 128] matrix doing the 7-tap vertical convolution.
    Bands are padded to 128 output rows so every matmul/evac/DMA touches
    all 128 partitions (uniform shapes; rows 122-127 are scratch).
  - DMA partition counts are all 128 for SDMA engine spray: the HWDGE
    splits a transfer's descriptors across (largest divisor of the
    partition count <= 16) engines at ~27 GB/s each; 122 = 2*61 would
    serialize on 2 engines.
  - Output DRAM is scratch-padded to [34*128, 512] so each tile is one
    full 128-partition write; the host gathers the valid 122 rows per
    tile. Outputs issue on the Activation HWDGE ring (nc.scalar), inputs
    on the SP ring (nc.sync), halving the serial DMA-issue load per ring.
  - Output stays fp32 (write bandwidth has headroom; halves rounding err).
"""

import sys

sys.path.insert(0, "/opt/trn_rl_repo")

import numpy as np

import concourse.bass as bass
import concourse.bacc as bacc
import concourse.mybir as mybir
from concourse.tile import TileContext
from concourse.bass_utils import run_bass_kernel_spmd

KH, KW = 7, 7
H, W = 4096, 4096
OH, OW = H - KH + 1, W - KW + 1  # 4090, 4090

NCORES = 8
CORE_OC = 512                     # output cols per core (core 7: 506 valid)
CORE_IC = CORE_OC + KW - 1        # 518 input cols per core
TILE_R = 122                      # valid output rows per tile
MROWS = 128                       # psum/output rows per tile (122 + 6 scratch)
N_TILES = -(-OH // TILE_R)        # 34 (33x122 + 64)

_NC_CACHE = {}


def _build_nc():
    f32 = mybir.dt.float32
    bf16 = mybir.dt.bfloat16

    nc = bacc.Bacc()
    x_in = nc.declare_dram_parameter("x_in", [H, CORE_IC], bf16, isOutput=False)
    bands = nc.declare_dram_parameter("bands", [128, KW * MROWS], bf16, isOutput=False)
    biasb = nc.declare_dram_parameter("biasb", [128, 1], f32, isOutput=False)
    y_out = nc.declare_dram_parameter(
        "y_out", [N_TILES * MROWS, CORE_OC], f32, isOutput=True
    )

    with TileContext(nc) as tc:
        with (
            tc.tile_pool(name="const", bufs=1) as cpool,
            tc.tile_pool(name="io", bufs=6) as iopool,
            tc.tile_pool(name="yo", bufs=4) as ypool,
            tc.tile_pool(name="ps", bufs=6, space="PSUM") as ppool,
        ):
            band_sb = cpool.tile([128, KW * MROWS], bf16)
            bias_sb = cpool.tile([128, 1], f32)
            nc.sync.dma_start(out=band_sb[:, :], in_=bands[:, :])
            nc.sync.dma_start(out=bias_sb[:, :], in_=biasb[:, :])

            for t in range(N_TILES):
                r0 = t * TILE_R
                kh = min(128, H - r0)
                x_sb = iopool.tile([128, CORE_IC], bf16, tag="x")
                nc.sync.dma_start(out=x_sb[:kh, :], in_=x_in[r0 : r0 + kh, :])
                ps = ppool.tile([128, CORE_OC], f32, tag="ps")
                for b in range(KW):
                    nc.tensor.matmul(
                        ps[:, :],
                        lhsT=band_sb[:kh, b * MROWS : (b + 1) * MROWS],
                        rhs=x_sb[:kh, b : b + CORE_OC],
                        start=(b == 0),
                        stop=(b == KW - 1),
                    )
                y_sb = ypool.tile([128, CORE_OC], f32, tag="y")
                nc.vector.tensor_scalar_add(y_sb[:, :], ps[:, :], bias_sb[:, 0:1])
                nc.scalar.dma_start(
                    out=y_out[t * MROWS : (t + 1) * MROWS, :], in_=y_sb[:, :]
                )
    nc.compile()
    return nc


def _make_bands(weight):
    """B_b[k, m] = w[k-m, b] laid out as [128, KW*MROWS] (band b in cols
    [b*MROWS, (b+1)*MROWS)); columns m >= TILE_R stay zero (scratch rows)."""
    bands = np.zeros((128, KW * MROWS), np.float32)
    m = np.arange(TILE_R)
    for b in range(KW):
        for a in range(KH):
            bands[m + a, b * MROWS + m] = weight[a, b]
    return bands.astype(mybir.dt.np(mybir.dt.bfloat16))


def _shard_inputs(x, weight, bias):
    bands = _make_bands(weight)
    biasb = np.full((128, 1), np.float32(bias[0]), np.float32)
    xb = x.astype(mybir.dt.np(mybir.dt.bfloat16))
    in_maps = []
    for c in range(NCORES):
        c0 = c * CORE_OC
        cc = min(CORE_IC, W - c0)
        xt = np.zeros((H, CORE_IC), xb.dtype)
        xt[:, :cc] = xb[:, c0 : c0 + cc]
        in_maps.append({"x_in": xt, "bands": bands, "biasb": biasb})
    return in_maps


def _assemble(results):
    out = np.empty((OH, OW), np.float32)
    for c in range(NCORES):
        c0 = c * CORE_OC
        cc = min(CORE_OC, OW - c0)
        yc = results[c]["y_out"]
        for t in range(N_TILES):
            r0 = t * TILE_R
            h = min(TILE_R, OH - r0)
            out[r0 : r0 + h, c0 : c0 + cc] = yc[t * MROWS : t * MROWS + h, :cc]
    return out


def _get_nc():
    if "nc" not in _NC_CACHE:
        _NC_CACHE["nc"] = _build_nc()
    return _NC_CACHE["nc"]


def _run(x, weight, bias, **spmd_kwargs):
    x = np.ascontiguousarray(np.asarray(x), dtype=np.float32)
    weight = np.asarray(weight, dtype=np.float32)
    bias = np.asarray(bias, dtype=np.float32)
    in_maps = _shard_inputs(x, weight, bias)
    res = run_bass_kernel_spmd(_get_nc(), in_maps, list(range(NCORES)), **spmd_kwargs)
    return _assemble(res.results), res


def kernel(x, weight, bias):
    out, _ = _run(x, weight, bias)
    return out


# revision 5
# speedup vs baseline: 2.7629x; 1.0045x over previous
"""Trainium2 Bass kernel: 7x7 valid cross-correlation (Conv2D) + bias on a
4096x4096 fp32 image, column-sharded over 8 NeuronCores (512 output columns
each, with a 6-column halo in each core's input slice).

Design (v3):
  - Column sharding: each core sees all 4096 input rows x 518 cols. Row
    tiles of 122 output rows (kin=128 input rows) give 34 tiles with only
    1.4% quantization waste (vs 19% for a row-sharded 512/122 split).
  - Input is cast to bf16 host-side: halves input HBM traffic and the PE
    streams bf16 at 1 col/cycle with no on-device cast.
  - The 2D conv is 7 accumulating matmuls per tile (one per horizontal tap
    b): psum[m, n] += B_b.T @ x[:, n+b], with B_b[k, m] = w[k-m, b] a
    banded [128 x 128] matrix doing the 7-tap vertical convolution.
    Bands are padded to 128 output rows so every matmul/evac/DMA touches
    all 128 partitions (uniform shapes; rows 122-127 are scratch).
  - DMA partition counts are all 128 for SDMA engine spray: the HWDGE
    splits a transfer's descriptors across (largest divisor of the
    partition count <= 16) engines at ~27 GB/s each; 122 = 2*61 would
    serialize on 2 engines.
  - Output DRAM is scratch-padded to [34*128, 512] so each tile is one
    full 128-partition write; the host gathers the valid 122 rows per
    tile. Outputs issue on the Activation HWDGE ring (nc.scalar), inputs
    on the SP ring (nc.sync), halving the serial DMA-issue load per ring.
  - Output stays fp32 (write bandwidth has headroom; halves rounding err).
"""

import sys

sys.path.insert(0, "/opt/trn_rl_repo")

import numpy as np

import concourse.bass as bass
import concourse.bacc as bacc
import concourse.mybir as mybir
from concourse.tile import TileContext
from concourse.bass_utils import run_bass_kernel_spmd

KH, KW = 7, 7
H, W = 4096, 4096
OH, OW = H - KH + 1, W - KW + 1  # 4090, 4090

NCORES = 8
CORE_OC = 512                     # output cols per core (core 7: 506 valid)
CORE_IC = CORE_OC + KW - 1        # 518 input cols per core
TILE_R = 122                      # valid output rows per tile
MROWS = 128                       # psum/output rows per tile (122 + 6 scratch)
N_TILES = -(-OH // TILE_R)        # 34 (33x122 + 64)

_NC_CACHE = {}


def _build_nc():
    f32 = mybir.dt.float32
    bf16 = mybir.dt.bfloat16

    nc = bacc.Bacc()
    x_in = nc.declare_dram_parameter("x_in", [H, CORE_IC], bf16, isOutput=False)
    bands = nc.declare_dram_parameter("bands", [128, KW * MROWS], bf16, isOutput=False)
    biasb = nc.declare_dram_parameter("biasb", [128, 1], f32, isOutput=False)
    y_out = nc.declare_dram_parameter(
        "y_out", [N_TILES * MROWS, CORE_OC], f32, isOutput=True
    )

    with TileContext(nc) as tc:
        with (
            tc.tile_pool(name="const", bufs=1) as cpool,
            tc.tile_pool(name="io", bufs=6) as iopool,
            tc.tile_pool(name="yo", bufs=4) as ypool,
            tc.tile_pool(name="ps", bufs=6, space="PSUM") as ppool,
        ):
            band_sb = cpool.tile([128, KW * MROWS], bf16)
            bias_sb = cpool.tile([128, 1], f32)
            # band on the Activation HWDGE ring so it overlaps the first x
            # tile's load on the SP ring
            nc.scalar.dma_start(out=band_sb[:, :], in_=bands[:, :])
            nc.scalar.dma_start(out=bias_sb[:, :], in_=biasb[:, :])

            # Warm up the PE during the DMA preamble: HAM un-throttles
            # (1.2 -> 2.4 GHz) only after ~3.4us of sustained PE activity,
            # so burn idle preamble time on dummy matmuls (inputs are
            # whatever SBUF holds; the psum result is never read).
            warm_in = cpool.tile([128, CORE_OC], bf16)
            nc.gpsimd.memset(warm_in[:, :], 0)
            warm_ps = ppool.tile([128, CORE_OC], f32, tag="ps")
            for _ in range(10):
                nc.tensor.matmul(
                    warm_ps[:, :],
                    lhsT=warm_in[:, :128],
                    rhs=warm_in[:, :CORE_OC],
                    start=True,
                    stop=True,
                )

            for t in range(N_TILES):
                r0 = t * TILE_R
                kh = min(128, H - r0)
                x_sb = iopool.tile([128, CORE_IC], bf16, tag="x")
                nc.sync.dma_start(out=x_sb[:kh, :], in_=x_in[r0 : r0 + kh, :])
                ps = ppool.tile([128, CORE_OC], f32, tag="ps")
                for b in range(KW):
                    nc.tensor.matmul(
                        ps[:, :],
                        lhsT=band_sb[:kh, b * MROWS : (b + 1) * MROWS],
                        rhs=x_sb[:kh, b : b + CORE_OC],
                        start=(b == 0),
                        stop=(b == KW - 1),
                    )
                y_sb = ypool.tile([128, CORE_OC], f32, tag="y")
                nc.vector.tensor_scalar_add(y_sb[:, :], ps[:, :], bias_sb[:, 0:1])
                nc.scalar.dma_start(
                    out=y_out[t * MROWS : (t + 1) * MROWS, :], in_=y_sb[:, :]
                )
    nc.compile()
    return nc


def _make_bands(weight):
    """B_b[k, m] = w[k-m, b] laid out as [128, KW*MROWS] (band b in cols
    [b*MROWS, (b+1)*MROWS)); columns m >= TILE_R stay zero (scratch rows)."""
    bands = np.zeros((128, KW * MROWS), np.float32)
    m = np.arange(TILE_R)
    for b in range(KW):
        for a in range(KH):
            bands[m + a, b * MROWS + m] = weight[a, b]
    return bands.astype(mybir.dt.np(mybir.dt.bfloat16))


def _shard_inputs(x, weight, bias):
    bands = _make_bands(weight)
    biasb = np.full((128, 1), np.float32(bias[0]), np.float32)
    xb = x.astype(mybir.dt.np(mybir.dt.bfloat16))
    in_maps = []
    for c in range(NCORES):
        c0 = c * CORE_OC
        cc = min(CORE_IC, W - c0)
        xt = np.zeros((H, CORE_IC), xb.dtype)
        xt[:, :cc] = xb[:, c0 : c0 + cc]
        in_maps.append({"x_in": xt, "bands": bands, "biasb": biasb})
    return in_maps


def _assemble(results):
    out = np.empty((OH, OW), np.float32)
    for c in range(NCORES):
        c0 = c * CORE_OC
        cc = min(CORE_OC, OW - c0)
        yc = results[c]["y_out"]
        for t in range(N_TILES):
            r0 = t * TILE_R
            h = min(TILE_R, OH - r0)
            out[r0 : r0 + h, c0 : c0 + cc] = yc[t * MROWS : t * MROWS + h, :cc]
    return out


def _get_nc():
    if "nc" not in _NC_CACHE:
        _NC_CACHE["nc"] = _build_nc()
    return _NC_CACHE["nc"]


def _run(x, weight, bias, **spmd_kwargs):
    x = np.ascontiguousarray(np.asarray(x), dtype=np.float32)
    weight = np.asarray(weight, dtype=np.float32)
    bias = np.asarray(bias, dtype=np.float32)
    in_maps = _shard_inputs(x, weight, bias)
    res = run_bass_kernel_spmd(_get_nc(), in_maps, list(range(NCORES)), **spmd_kwargs)
    return _assemble(res.results), res


def kernel(x, weight, bias):
    out, _ = _run(x, weight, bias)
    return out
